# revision 1
# baseline (speedup 1.0000x reference)
"""GNN message-passing layer on 8 Trainium2 NeuronCores.

Strategy: edges are bucketed by destination node (6250 nodes/core), so the
segment-sum is core-local and no collectives are needed.

Per core:
  phase 1: A|B tables  A = nf @ Wm1[0:64] + b_m1,  B = nf @ Wm1[64:128]
           (node-major in DRAM, computed from host-transposed node_feat)
  phase 2: per 128-node block (49 blocks x 22 tiles x 128 edges, padded):
           gather A[src], B[dst] (indirect DMA), EF @ Wm1[128:192] on PE
           (host-pretransposed edge_feat tiles as stationary), add + SiLU,
           then segment-sum via one-hot matmul accumulating into PSUM.
           The one-hot is built on DVE from within-block dst indices.
  phase 3: aggregated = Wm2^T @ H + b_m2*deg (deg from host bincount),
           update MLP feature-major (biases are per-partition), residual,
           LayerNorm via PE ones-matmul stats + K=1 broadcast matmuls.

Output is written feature-major [64, 6272] per core; host transposes and
concatenates.
"""
import sys
sys.path.insert(0, "/opt/trn_rl_repo")
import numpy as np

import concourse.bass as bass
import concourse.bacc as bacc
import concourse.mybir as mybir
import concourse.tile as tile
from concourse.bass_utils import run_bass_kernel_spmd

F32 = mybir.dt.float32
I32 = mybir.dt.int32

N_NODES = 50000
N_EDGES = 1000000
D = 64
NC = 8
NPC = 6250            # nodes per core
BLOCKS = 49           # 49 * 128 = 6272 local node slots
TPB = 22              # tiles (of 128 edges) per block
NLOC = BLOCKS * 128   # 6272
TILES = BLOCKS * TPB  # 1078
EPC = TILES * 128     # 137984 padded edges per core
NPAD = 50176          # 392 * 128 node slots for the A/B table
LN_EPS = 1e-5

_CACHED = {}


def _build_bass():
    nc = bacc.Bacc("TRN2", target_bir_lowering=False, debug=False, num_devices=NC)

    # ---- I/O ----
    nf_t = nc.dram_tensor("nf_t", [65, NPAD], F32, kind="ExternalInput")
    nf_loc = nc.dram_tensor("nf_loc", [D, NLOC], F32, kind="ExternalInput")
    ef_t = nc.dram_tensor("ef_t", [TILES, D, 128], F32, kind="ExternalInput")
    isrc = nc.dram_tensor("isrc", [128, TILES], I32, kind="ExternalInput")
    idst = nc.dram_tensor("idst", [128, TILES], I32, kind="ExternalInput")
    dstr = nc.dram_tensor("dstr", [128, TILES], F32, kind="ExternalInput")
    deg = nc.dram_tensor("deg", [1, NLOC], F32, kind="ExternalInput")
    iota = nc.dram_tensor("iota", [128, 128], F32, kind="ExternalInput")
    w_ab = nc.dram_tensor("w_ab", [65, 128], F32, kind="ExternalInput")
    w_e = nc.dram_tensor("w_e", [D, D], F32, kind="ExternalInput")
    w_m2e = nc.dram_tensor("w_m2e", [65, D], F32, kind="ExternalInput")
    w_u1 = nc.dram_tensor("w_u1", [D, D], F32, kind="ExternalInput")
    b_u1 = nc.dram_tensor("b_u1", [D, 1], F32, kind="ExternalInput")
    w_u2e = nc.dram_tensor("w_u2e", [65, D], F32, kind="ExternalInput")
    gam = nc.dram_tensor("gam", [D, 1], F32, kind="ExternalInput")
    bet = nc.dram_tensor("bet", [D, 1], F32, kind="ExternalInput")
    out_fm = nc.dram_tensor("out_fm", [D, NLOC], F32, kind="ExternalOutput")

    with tile.TileContext(nc) as tc:
        with (
            tc.tile_pool(name="dram", bufs=1, space="DRAM") as dpool,
            tc.tile_pool(name="persist", bufs=1) as pp,
        ):
            a_t = dpool.tile([NPAD, D], F32, tag="a_t")
            b_t = dpool.tile([NPAD, D], F32, tag="b_t")

            # persistent SBUF state
            isrc_sb = pp.tile([128, TILES], I32, tag="isrc")
            idst_sb = pp.tile([128, TILES], I32, tag="idst")
            dstr_sb = pp.tile([128, TILES], F32, tag="dstr")
            iota_sb = pp.tile([128, 128], F32, tag="iota")
            wab_sb = pp.tile([65, 128], F32, tag="wab")
            we_sb = pp.tile([D, D], F32, tag="we")
            wm2_sb = pp.tile([65, D], F32, tag="wm2")
            wu1_sb = pp.tile([D, D], F32, tag="wu1")
            bu1_sb = pp.tile([D, 1], F32, tag="bu1")
            wu2_sb = pp.tile([65, D], F32, tag="wu2")
            gam_sb = pp.tile([D, 1], F32, tag="gam")
            bet_sb = pp.tile([D, 1], F32, tag="bet")
            hall = pp.tile([65, NLOC], F32, tag="hall")
            oinv = pp.tile([D, 1], F32, tag="oinv")   # 1/64 column
            eps_sb = pp.tile([1, 1], F32, tag="eps")
            ones1 = pp.tile([1, D], F32, tag="ones1")  # row of 1.0

            nc.sync.dma_start(isrc_sb[:], isrc[:])
            nc.sync.dma_start(idst_sb[:], idst[:])
            nc.sync.dma_start(dstr_sb[:], dstr[:])
            nc.sync.dma_start(iota_sb[:], iota[:])
            nc.sync.dma_start(wab_sb[:], w_ab[:])
            nc.sync.dma_start(we_sb[:], w_e[:])
            nc.sync.dma_start(wm2_sb[:], w_m2e[:])
            nc.sync.dma_start(wu1_sb[:], w_u1[:])
            nc.sync.dma_start(bu1_sb[:], b_u1[:])
            nc.sync.dma_start(wu2_sb[:], w_u2e[:])
            nc.sync.dma_start(gam_sb[:], gam[:])
            nc.sync.dma_start(bet_sb[:], bet[:])
            nc.sync.dma_start(hall[64:65, :], deg[:])
            nc.gpsimd.memset(oinv[:], 1.0 / 64.0)
            nc.gpsimd.memset(eps_sb[:], LN_EPS)
            nc.gpsimd.memset(ones1[:], 1.0)

            # ---------------- phase 1: A/B tables ----------------
            with (
                tc.tile_pool(name="p1", bufs=3) as p1,
                tc.tile_pool(name="p1ps", bufs=2, space="PSUM") as p1ps,
            ):
                for nb in range(NPAD // 128):
                    sl = slice(nb * 128, (nb + 1) * 128)
                    nfc = p1.tile([65, 128], F32, tag="nfc")
                    nc.sync.dma_start(nfc[:], nf_t[:, sl])
                    ps = p1ps.tile([128, 128], F32, tag="ab")
                    nc.tensor.matmul(ps[:], lhsT=nfc[:], rhs=wab_sb[:],
                                     start=True, stop=True)
                    ab = p1.tile([128, 128], F32, tag="ab_sb")
                    nc.vector.tensor_copy(ab[:], ps[:])
                    nc.sync.dma_start(a_t[:].rearrange("n d -> n d")[sl, :], ab[:, 0:D])
                    nc.sync.dma_start(b_t[:][sl, :], ab[:, D:2 * D])

            # ---------------- phase 2: edges ----------------
            with (
                tc.tile_pool(name="p2", bufs=3) as p2,
                tc.tile_pool(name="p2b", bufs=2) as p2b,
                tc.tile_pool(name="p2ps", bufs=2, space="PSUM") as p2ps,
                tc.tile_pool(name="p2ph", bufs=2, space="PSUM") as p2ph,
            ):
                for b in range(BLOCKS):
                    tsl = slice(b * TPB, (b + 1) * TPB)
                    ga = p2.tile([128, TPB * D], F32, tag="ga")
                    gb = p2.tile([128, TPB * D], F32, tag="gb")
                    for t in range(TPB):
                        gt = b * TPB + t
                        nc.gpsimd.indirect_dma_start(
                            out=ga[:, t * D:(t + 1) * D], out_offset=None,
                            in_=a_t[:],
                            in_offset=bass.IndirectOffsetOnAxis(
                                ap=isrc_sb[:, gt:gt + 1], axis=0),
                        )
                        nc.gpsimd.indirect_dma_start(
                            out=gb[:, t * D:(t + 1) * D], out_offset=None,
                            in_=b_t[:],
                            in_offset=bass.IndirectOffsetOnAxis(
                                ap=idst_sb[:, gt:gt + 1], axis=0),
                        )
                    nc.vector.tensor_add(ga[:], ga[:], gb[:])

                    ef = p2.tile([D, TPB * 128], F32, tag="ef")
                    nc.sync.dma_start(
                        ef[:].rearrange("f (t x) -> f t x", t=TPB),
                        ef_t[tsl, :, :].rearrange("t f x -> f t x"),
                    )

                    oh = p2b.tile([128, TPB * 128], F32, tag="oh")
                    nc.vector.tensor_tensor(
                        out=oh[:].rearrange("p (t x) -> p t x", t=TPB),
                        in0=iota_sb[:, None, :].to_broadcast([128, TPB, 128]),
                        in1=dstr_sb[:, tsl, None].to_broadcast([128, TPB, 128]),
                        op=mybir.AluOpType.is_equal,
                    )

                    pre = p2b.tile([128, TPB * D], F32, tag="pre")
                    # EF matmuls in groups of 8 tiles -> one PSUM bank
                    for g in range((TPB + 7) // 8):
                        t0, t1 = g * 8, min((g + 1) * 8, TPB)
                        ps8 = p2ps.tile([128, 512], F32, tag="ps8")
                        for t in range(t0, t1):
                            nc.tensor.matmul(
                                ps8[:, (t - t0) * D:(t - t0 + 1) * D],
                                lhsT=ef[:, t * 128:(t + 1) * 128],
                                rhs=we_sb[:],
                                start=True, stop=True, skip_group_check=True,
                            )
                        nc.vector.tensor_add(
                            pre[:, t0 * D:t1 * D],
                            ps8[:, 0:(t1 - t0) * D],
                            ga[:, t0 * D:t1 * D],
                        )
                    h = p2b.tile([128, TPB * D], F32, tag="h")
                    nc.scalar.activation(h[:], pre[:],
                                         mybir.ActivationFunctionType.Silu)

                    psH = p2ph.tile([D, 128], F32, tag="psH")
                    for t in range(TPB):
                        nc.tensor.matmul(
                            psH[:],
                            lhsT=h[:, t * D:(t + 1) * D],
                            rhs=oh[:, t * 128:(t + 1) * 128],
                            start=(t == 0), stop=(t == TPB - 1),
                        )
                    nc.vector.tensor_copy(hall[0:D, b * 128:(b + 1) * 128], psH[:])

            # ---------------- phase 3: update MLP + LayerNorm ----------------
            with (
                tc.tile_pool(name="p3", bufs=2) as p3,
                tc.tile_pool(name="p3ps", bufs=1, space="PSUM") as p3ps,
            ):
                starts = list(range(0, NLOC, 512))
                for cs in starts:
                    w = min(512, NLOC - cs)
                    sl = slice(cs, cs + w)
                    ps_a = p3ps.tile([D, 512], F32, tag="ps_a")
                    nc.tensor.matmul(ps_a[:, :w], lhsT=wm2_sb[:], rhs=hall[:, sl],
                                     start=True, stop=True)
                    agg = p3.tile([D, 512], F32, tag="agg")
                    nc.vector.tensor_copy(agg[:, :w], ps_a[:, :w])

                    ps_u1 = p3ps.tile([D, 512], F32, tag="ps_u1")
                    nc.tensor.matmul(ps_u1[:, :w], lhsT=wu1_sb[:], rhs=agg[:, :w],
                                     start=True, stop=True)
                    s1 = p3.tile([65, 512], F32, tag="s1")
                    nc.gpsimd.memset(s1[64:65, :w], 1.0)
                    nc.scalar.activation(s1[0:D, :w], ps_u1[:, :w],
                                         mybir.ActivationFunctionType.Silu,
                                         bias=bu1_sb[:])
                    ps_u2 = p3ps.tile([D, 512], F32, tag="ps_u2")
                    nc.tensor.matmul(ps_u2[:, :w], lhsT=wu2_sb[:], rhs=s1[:, :w],
                                     start=True, stop=True)

                    nfl = p3.tile([D, 512], F32, tag="nfl")
                    nc.sync.dma_start(nfl[:, :w], nf_loc[:, sl])
                    xr = p3.tile([D, 512], F32, tag="xr")
                    nc.vector.tensor_add(xr[:, :w], ps_u2[:, :w], nfl[:, :w])

                    sq = p3.tile([D, 512], F32, tag="sq")
                    nc.scalar.activation(sq[:, :w], xr[:, :w],
                                         mybir.ActivationFunctionType.Square)
                    ps_s1 = p3ps.tile([1, 512], F32, tag="ps_s1")
                    nc.tensor.matmul(ps_s1[:, :w], lhsT=oinv[:], rhs=xr[:, :w],
                                     start=True, stop=True)
                    ps_s2 = p3ps.tile([1, 512], F32, tag="ps_s2")
                    nc.tensor.matmul(ps_s2[:, :w], lhsT=oinv[:], rhs=sq[:, :w],
                                     start=True, stop=True)
                    mean_sb = p3.tile([1, 512], F32, tag="mean_sb")
                    nc.vector.tensor_copy(mean_sb[:, :w], ps_s1[:, :w])
                    msq = p3.tile([1, 512], F32, tag="msq")
                    nc.vector.tensor_mul(msq[:, :w], mean_sb[:, :w], mean_sb[:, :w])
                    var = p3.tile([1, 512], F32, tag="var")
                    nc.vector.tensor_tensor(out=var[:, :w], in0=ps_s2[:, :w],
                                            in1=msq[:, :w],
                                            op=mybir.AluOpType.subtract)
                    std = p3.tile([1, 512], F32, tag="std")
                    nc.scalar.activation(std[:, :w], var[:, :w],
                                         mybir.ActivationFunctionType.Sqrt,
                                         bias=eps_sb[:])
                    rstd = p3.tile([1, 512], F32, tag="rstd")
                    nc.vector.reciprocal(rstd[:, :w], std[:, :w])

                    ps_mb = p3ps.tile([D, 512], F32, tag="ps_mb")
                    nc.tensor.matmul(ps_mb[:, :w], lhsT=ones1[:], rhs=mean_sb[:, :w],
                                     start=True, stop=True)
                    ps_rb = p3ps.tile([D, 512], F32, tag="ps_rb")
                    nc.tensor.matmul(ps_rb[:, :w], lhsT=ones1[:], rhs=rstd[:, :w],
                                     start=True, stop=True)

                    t1_ = p3.tile([D, 512], F32, tag="t1")
                    nc.vector.tensor_tensor(out=t1_[:, :w], in0=xr[:, :w],
                                            in1=ps_mb[:, :w],
                                            op=mybir.AluOpType.subtract)
                    t2_ = p3.tile([D, 512], F32, tag="t2")
                    nc.vector.tensor_mul(t2_[:, :w], t1_[:, :w], ps_rb[:, :w])
                    oc = p3.tile([D, 512], F32, tag="oc")
                    nc.scalar.activation(oc[:, :w], t2_[:, :w],
                                         mybir.ActivationFunctionType.Identity,
                                         bias=bet_sb[:], scale=gam_sb[:])
                    nc.sync.dma_start(out_fm[:, sl], oc[:, :w])

    nc.compile()
    return nc


def _prep(node_feat, edge_src, edge_dst, edge_feat,
          W_m1, b_m1, W_m2, b_m2, W_u1, b_u1, W_u2, b_u2,
          ln_gamma, ln_beta):
    """Host-side sharding: bucket+sort edges by dst, pad to fixed tiles."""
    order = np.argsort(edge_dst, kind="stable")
    sdst = edge_dst[order]

    nf_t = np.zeros((65, NPAD), np.float32)
    nf_t[0:D, 0:N_NODES] = node_feat.T
    nf_t[64, :] = 1.0

    w_ab = np.zeros((65, 128), np.float32)
    w_ab[0:D, 0:D] = W_m1[0:D]
    w_ab[0:D, D:2 * D] = W_m1[D:2 * D]
    w_ab[64, 0:D] = b_m1
    w_e = np.ascontiguousarray(W_m1[2 * D:3 * D])
    w_m2e = np.zeros((65, D), np.float32)
    w_m2e[0:D] = W_m2
    w_m2e[64] = b_m2
    w_u2e = np.zeros((65, D), np.float32)
    w_u2e[0:D] = W_u2
    w_u2e[64] = b_u2
    iota = np.tile(np.arange(128, dtype=np.float32), (128, 1))

    common = {
        "nf_t": nf_t, "iota": iota, "w_ab": w_ab, "w_e": w_e,
        "w_m2e": w_m2e, "w_u1": np.ascontiguousarray(W_u1),
        "b_u1": b_u1.reshape(D, 1).astype(np.float32), "w_u2e": w_u2e,
        "gam": ln_gamma.reshape(D, 1).astype(np.float32),
        "bet": ln_beta.reshape(D, 1).astype(np.float32),
    }

    in_maps = []
    for c in range(NC):
        lo, hi = c * NPC, (c + 1) * NPC
        e0, e1 = np.searchsorted(sdst, lo), np.searchsorted(sdst, hi)
        eidx = order[e0:e1]
        ldst = sdst[e0:e1] - lo                     # local dst in [0, NPC)

        isrc = np.zeros((128, TILES), np.int32)
        idst = np.zeros((128, TILES), np.int32)
        dstr = np.full((128, TILES), -1.0, np.float32)
        ef_tiles = np.zeros((TILES, 128, D), np.float32)

        bstart = np.searchsorted(ldst, np.arange(BLOCKS + 1) * 128)
        for b in range(BLOCKS):
            n = bstart[b + 1] - bstart[b]
            if n > TPB * 128:
                raise ValueError(f"block overflow: core {c} block {b}: {n}")
            sel = eidx[bstart[b]:bstart[b + 1]]
            rel = (ldst[bstart[b]:bstart[b + 1]] - b * 128).astype(np.float32)
            # slot k within block -> tile b*TPB + k//128, partition k%128
            t_of = b * TPB + np.arange(n) // 128
            p_of = np.arange(n) % 128
            isrc[p_of, t_of] = edge_src[sel]
            idst[p_of, t_of] = edge_dst[sel]
            dstr[p_of, t_of] = rel
            ef_tiles[t_of, p_of] = edge_feat[sel]

        degc = np.zeros((1, NLOC), np.float32)
        cnt = np.bincount(ldst, minlength=NPC).astype(np.float32)
        degc[0, 0:NPC] = cnt

        nf_loc = np.zeros((D, NLOC), np.float32)
        nhi = min(N_NODES, lo + NLOC)
        nf_loc[:, 0:nhi - lo] = node_feat[lo:nhi].T

        in_maps.append({
            **common,
            "nf_loc": nf_loc,
            "ef_t": np.ascontiguousarray(ef_tiles.transpose(0, 2, 1)),
            "isrc": isrc, "idst": idst, "dstr": dstr, "deg": degc,
        })
    return in_maps


def kernel(**inputs):
    inputs = {k: np.asarray(v) for k, v in inputs.items()}
    in_maps = _prep(**inputs)
    if "nc" not in _CACHED:
        _CACHED["nc"] = _build_bass()
    res = run_bass_kernel_spmd(_CACHED["nc"], in_maps, list(range(NC)))
    out = np.empty((N_NODES, D), np.float32)
    for c in range(NC):
        out[c * NPC:(c + 1) * NPC] = res.results[c]["out_fm"].T[0:NPC]
    return out


if __name__ == "__main__":
    rng = np.random.default_rng(1)
    sys.path.insert(0, "/root/problem")
    import reference
    inputs = {k: np.asarray(v) for k, v in reference.setup_inputs().items()}
    exp = np.asarray(reference.reference(**inputs))
    got = kernel(**inputs)
    err = np.abs(got - exp).max() / (np.abs(exp).max() + 1e-30)
    print("Relative error:", err)



# revision 7
# speedup vs baseline: 1.4206x; 1.4206x over previous
"""GNN message-passing layer on 8 Trainium2 NeuronCores.

Strategy: edges are bucketed by destination node (6250 nodes/core), so the
segment-sum is core-local and no collectives are needed.

Per core (f16 data path, fp32 accumulation in PSUM / LayerNorm):
  phase 1:  A table  A = nf @ Wm1[0:64] + b_m1  -> DRAM [NPAD, 64] f32
  phase 1.5: local B table B = nf_loc @ Wm1[64:128] -> DRAM [NLOC, 64] f32
  phase 2:  per dst-block of 128 nodes, edges are split into "lo" tiles
            (src < 32768) and "hi" tiles (src >= 32768) so that batched
            dma_gather (int16 indices) can fetch A[src] for thousands of
            edges in one instruction; B[dst] is gathered from the local
            table (local dst < 6272 always fits int16). Superblocks of
            SBB dst blocks share one (ga_lo, ga_hi, gb) gather triple.
            Then EF @ Wm1[128:192] on PE, add + SiLU, segment-sum via
            one-hot matmul into PSUM (one-hot built on DVE).
  phase 3:  aggregated = Wm2^T @ H + b_m2*deg, update MLP feature-major,
            residual, LayerNorm via PE ones-matmul stats.

Output is written feature-major [64, 6272] per core; host transposes and
concatenates.
"""
import sys
sys.path.insert(0, "/opt/trn_rl_repo")
import numpy as np

import concourse.bass as bass
import concourse.bacc as bacc
import concourse.mybir as mybir
import concourse.tile as tile
from concourse.bass_utils import run_bass_kernel_spmd

F32 = mybir.dt.float32
F16 = mybir.dt.float16
I16 = mybir.dt.int16

N_NODES = 50000
N_EDGES = 1000000
D = 64
NC = 8
NPC = 6250            # nodes per core
BLOCKS = 49           # 49 * 128 = 6272 local node slots
NLOC = BLOCKS * 128   # 6272
NPAD = 50176          # 392 * 128 node slots for the A table
SPLIT = 32768         # int16 index range split for the A gather
SBB = 3               # dst blocks per gather superblock
LN_EPS = 1e-5

_CACHED = {}


def _build_bass(lo_t, hi_t):
    """lo_t/hi_t: per-block tile counts (length BLOCKS), shared by all cores."""
    nt = [l + h for l, h in zip(lo_t, hi_t)]
    T_tot = sum(nt)
    TB = np.concatenate([[0], np.cumsum(nt)]).astype(int)
    LO = np.concatenate([[0], np.cumsum(lo_t)]).astype(int)
    HI = np.concatenate([[0], np.cumsum(hi_t)]).astype(int)
    n_lo, n_hi = int(LO[-1]), int(HI[-1])
    sbs = [list(range(s, min(s + SBB, BLOCKS))) for s in range(0, BLOCKS, SBB)]
    MAXNT = max(nt)
    MAXLOSB = max(sum(lo_t[b] for b in sb) for sb in sbs)
    MAXHISB = max(sum(hi_t[b] for b in sb) for sb in sbs)
    MAXTSB = max(sum(nt[b] for b in sb) for sb in sbs)

    nc = bacc.Bacc("TRN2", target_bir_lowering=False, debug=False, num_devices=NC)

    # ---- I/O ----
    nf_tb = nc.dram_tensor("nf_tb", [65, NPAD], F16, kind="ExternalInput")
    nf_locb = nc.dram_tensor("nf_locb", [D, NLOC], F16, kind="ExternalInput")
    nf_loc = nc.dram_tensor("nf_loc", [D, NLOC], F32, kind="ExternalInput")
    ef_t = nc.dram_tensor("ef_t", [D, T_tot * 128], F16, kind="ExternalInput")
    idx_lo = nc.dram_tensor("idx_lo", [128, n_lo * 8], I16, kind="ExternalInput")
    idx_hi = nc.dram_tensor("idx_hi", [128, max(n_hi, 1) * 8], I16,
                            kind="ExternalInput")
    idx_b = nc.dram_tensor("idx_b", [128, T_tot * 8], I16, kind="ExternalInput")
    dstr = nc.dram_tensor("dstr", [128, T_tot], F16, kind="ExternalInput")
    deg = nc.dram_tensor("deg", [1, NLOC], F16, kind="ExternalInput")
    iota = nc.dram_tensor("iota", [128, 128], F16, kind="ExternalInput")
    w_a = nc.dram_tensor("w_a", [65, D], F16, kind="ExternalInput")
    w_b = nc.dram_tensor("w_b", [D, D], F16, kind="ExternalInput")
    w_e = nc.dram_tensor("w_e", [D, D], F16, kind="ExternalInput")
    w_m2e = nc.dram_tensor("w_m2e", [65, D], F16, kind="ExternalInput")
    w_u1 = nc.dram_tensor("w_u1", [D, D], F16, kind="ExternalInput")
    b_u1 = nc.dram_tensor("b_u1", [D, 1], F32, kind="ExternalInput")
    w_u2 = nc.dram_tensor("w_u2", [D, D], F16, kind="ExternalInput")
    b_u2 = nc.dram_tensor("b_u2", [D, 1], F32, kind="ExternalInput")
    gam = nc.dram_tensor("gam", [D, 1], F32, kind="ExternalInput")
    bet = nc.dram_tensor("bet", [D, 1], F32, kind="ExternalInput")
    out_fm = nc.dram_tensor("out_fm", [D, NLOC], F32, kind="ExternalOutput")

    with tile.TileContext(nc) as tc:
        with (
            tc.tile_pool(name="dram", bufs=1, space="DRAM") as dpool,
            tc.tile_pool(name="persist", bufs=1) as pp,
        ):
            a_t = dpool.tile([NPAD, D], F32, tag="a_t")
            b_loc = dpool.tile([NLOC, D], F32, tag="b_loc")

            # persistent SBUF state
            ilo_sb = pp.tile([128, n_lo * 8], I16, tag="ilo")
            ihi_sb = pp.tile([128, max(n_hi, 1) * 8], I16, tag="ihi")
            ib_sb = pp.tile([128, T_tot * 8], I16, tag="ib")
            dstr_sb = pp.tile([128, T_tot], F16, tag="dstr")
            iota_sb = pp.tile([128, 128], F16, tag="iota")
            wa_sb = pp.tile([65, D], F16, tag="wa")
            wb_sb = pp.tile([D, D], F16, tag="wb")
            we_sb = pp.tile([D, D], F16, tag="we")
            wm2_sb = pp.tile([65, D], F16, tag="wm2")
            wu1_sb = pp.tile([D, D], F16, tag="wu1")
            bu1_sb = pp.tile([D, 1], F32, tag="bu1")
            wu2_sb = pp.tile([D, D], F16, tag="wu2")
            bu2_sb = pp.tile([D, 1], F32, tag="bu2")
            gam_sb = pp.tile([D, 1], F32, tag="gam")
            bet_sb = pp.tile([D, 1], F32, tag="bet")
            hall = pp.tile([65, NLOC], F16, tag="hall")
            oinv = pp.tile([D, 1], F32, tag="oinv")   # 1/64 column
            eps_sb = pp.tile([1, 1], F32, tag="eps")
            ones1 = pp.tile([1, D], F32, tag="ones1")  # row of 1.0

            nc.sync.dma_start(ilo_sb[:], idx_lo[:])
            nc.sync.dma_start(ihi_sb[:], idx_hi[:])
            nc.sync.dma_start(ib_sb[:], idx_b[:])
            nc.sync.dma_start(dstr_sb[:], dstr[:])
            nc.sync.dma_start(iota_sb[:], iota[:])
            nc.sync.dma_start(wa_sb[:], w_a[:])
            nc.sync.dma_start(wb_sb[:], w_b[:])
            nc.sync.dma_start(we_sb[:], w_e[:])
            nc.sync.dma_start(wm2_sb[:], w_m2e[:])
            nc.sync.dma_start(wu1_sb[:], w_u1[:])
            nc.sync.dma_start(bu1_sb[:], b_u1[:])
            nc.sync.dma_start(wu2_sb[:], w_u2[:])
            nc.sync.dma_start(bu2_sb[:], b_u2[:])
            nc.sync.dma_start(gam_sb[:], gam[:])
            nc.sync.dma_start(bet_sb[:], bet[:])
            nc.sync.dma_start(hall[64:65, :], deg[:])
            nc.gpsimd.memset(oinv[:], 1.0 / 64.0)
            nc.gpsimd.memset(eps_sb[:], LN_EPS)
            nc.gpsimd.memset(ones1[:], 1.0)

            # ---------------- phase 1: A table ----------------
            with (
                tc.tile_pool(name="p1", bufs=3) as p1,
                tc.tile_pool(name="p1ps", bufs=2, space="PSUM") as p1ps,
            ):
                for g in range(NPAD // 1024):
                    nfc = p1.tile([65, 1024], F16, tag="nfc")
                    nc.sync.dma_start(nfc[:], nf_tb[:, g * 1024:(g + 1) * 1024])
                    ps = p1ps.tile([128, 512], F32, tag="aps")
                    for c in range(8):
                        nc.tensor.matmul(
                            ps[:, c * 64:(c + 1) * 64],
                            lhsT=nfc[:, c * 128:(c + 1) * 128],
                            rhs=wa_sb[:],
                            start=True, stop=True, skip_group_check=True,
                        )
                    stage = p1.tile([128, 512], F32, tag="stage")
                    nc.vector.tensor_copy(stage[:], ps[:])
                    nc.sync.dma_start(
                        a_t[:][g * 1024:(g + 1) * 1024, :]
                        .rearrange("(c p) f -> p c f", p=128),
                        stage[:].rearrange("p (c f) -> p c f", c=8),
                    )

                # ---- phase 1.5: local B table (49 chunks of 128 nodes) ----
                for g in range(7):          # 7 groups of 7 chunks = 49
                    nflb = p1.tile([D, 896], F16, tag="nflb")
                    nc.sync.dma_start(nflb[:], nf_locb[:, g * 896:(g + 1) * 896])
                    ps = p1ps.tile([128, 512], F32, tag="aps")
                    for c in range(7):
                        nc.tensor.matmul(
                            ps[:, c * 64:(c + 1) * 64],
                            lhsT=nflb[:, c * 128:(c + 1) * 128],
                            rhs=wb_sb[:],
                            start=True, stop=True, skip_group_check=True,
                        )
                    stage = p1.tile([128, 512], F32, tag="stage")
                    nc.vector.tensor_copy(stage[:, 0:7 * 64], ps[:, 0:7 * 64])
                    nc.sync.dma_start(
                        b_loc[:][g * 896:(g + 1) * 896, :]
                        .rearrange("(c p) f -> p c f", p=128),
                        stage[:, 0:7 * 64].rearrange("p (c f) -> p c f", c=7),
                    )

            # ---------------- phase 2: edges ----------------
            with (
                tc.tile_pool(name="p2", bufs=2) as p2,
                tc.tile_pool(name="p2b", bufs=2) as p2b,
                tc.tile_pool(name="p2ps", bufs=2, space="PSUM") as p2ps,
                tc.tile_pool(name="p2ph", bufs=2, space="PSUM") as p2ph,
            ):
                for sb in sbs:
                    b0 = sb[0]
                    nlo_sb = sum(lo_t[b] for b in sb)
                    nhi_sb = sum(hi_t[b] for b in sb)
                    nt_sb = sum(nt[b] for b in sb)
                    ga_lo = p2.tile([128, MAXLOSB * 64], F32, tag="ga_lo")
                    nc.gpsimd.dma_gather(
                        ga_lo[:, 0:nlo_sb * 64].rearrange(
                            "p (c f) -> p c f", c=nlo_sb),
                        a_t[:][0:SPLIT, :],
                        ilo_sb[:, LO[b0] * 8:(LO[b0] + nlo_sb) * 8],
                        nlo_sb * 128, nlo_sb * 128, 64,
                        single_packet=False,
                    )
                    ga_hi = p2.tile([128, max(MAXHISB, 1) * 64], F32, tag="ga_hi")
                    if nhi_sb:
                        nc.gpsimd.dma_gather(
                            ga_hi[:, 0:nhi_sb * 64].rearrange(
                                "p (c f) -> p c f", c=nhi_sb),
                            a_t[:][SPLIT:NPAD, :],
                            ihi_sb[:, HI[b0] * 8:(HI[b0] + nhi_sb) * 8],
                            nhi_sb * 128, nhi_sb * 128, 64,
                            single_packet=False,
                        )
                    gb = p2.tile([128, MAXTSB * 64], F32, tag="gb")
                    nc.gpsimd.dma_gather(
                        gb[:, 0:nt_sb * 64].rearrange(
                            "p (c f) -> p c f", c=nt_sb),
                        b_loc[:],
                        ib_sb[:, TB[b0] * 8:(TB[b0] + nt_sb) * 8],
                        nt_sb * 128, nt_sb * 128, 64,
                        single_packet=False,
                    )

                    for b in sb:
                        lt, ht, ntb = lo_t[b], hi_t[b], nt[b]
                        lo_o = (LO[b] - LO[b0]) * 64       # cols into ga_lo
                        hi_o = (HI[b] - HI[b0]) * 64
                        bt_o = (TB[b] - TB[b0]) * 64       # cols into gb
                        gab = p2b.tile([128, MAXNT * 64], F16, tag="gab")
                        nc.vector.tensor_add(
                            gab[:, 0:lt * 64],
                            ga_lo[:, lo_o:lo_o + lt * 64],
                            gb[:, bt_o:bt_o + lt * 64])
                        if ht:
                            nc.vector.tensor_add(
                                gab[:, lt * 64:ntb * 64],
                                ga_hi[:, hi_o:hi_o + ht * 64],
                                gb[:, bt_o + lt * 64:bt_o + ntb * 64])

                        ef = p2b.tile([64, MAXNT * 128], F16, tag="ef")
                        nc.sync.dma_start(
                            ef[:, 0:ntb * 128],
                            ef_t[:, TB[b] * 128:(TB[b] + ntb) * 128])

                        oh = p2b.tile([128, MAXNT * 128], F16, tag="oh")
                        nc.vector.tensor_tensor(
                            out=oh[:, 0:ntb * 128].rearrange(
                                "p (t x) -> p t x", t=ntb),
                            in0=iota_sb[:, None, :].to_broadcast([128, ntb, 128]),
                            in1=dstr_sb[:, TB[b]:TB[b] + ntb, None]
                            .to_broadcast([128, ntb, 128]),
                            op=mybir.AluOpType.is_equal,
                        )

                        pre = p2b.tile([128, MAXNT * 64], F16, tag="pre")
                        for g in range((ntb + 7) // 8):
                            t0, t1 = g * 8, min((g + 1) * 8, ntb)
                            ps8 = p2ps.tile([128, 512], F32, tag="ps8")
                            for t in range(t0, t1):
                                nc.tensor.matmul(
                                    ps8[:, (t - t0) * D:(t - t0 + 1) * D],
                                    lhsT=ef[:, t * 128:(t + 1) * 128],
                                    rhs=we_sb[:],
                                    start=True, stop=True, skip_group_check=True,
                                )
                            nc.vector.tensor_add(
                                pre[:, t0 * D:t1 * D],
                                ps8[:, 0:(t1 - t0) * D],
                                gab[:, t0 * D:t1 * D],
                            )
                        h = p2b.tile([128, MAXNT * 64], F16, tag="h")
                        nc.scalar.activation(h[:, 0:ntb * 64], pre[:, 0:ntb * 64],
                                             mybir.ActivationFunctionType.Silu)

                        psH = p2ph.tile([D, 128], F32, tag="psH")
                        for t in range(ntb):
                            nc.tensor.matmul(
                                psH[:],
                                lhsT=h[:, t * D:(t + 1) * D],
                                rhs=oh[:, t * 128:(t + 1) * 128],
                                start=(t == 0), stop=(t == ntb - 1),
                            )
                        nc.vector.tensor_copy(
                            hall[0:D, b * 128:(b + 1) * 128], psH[:])

            # ---------------- phase 3: update MLP + LayerNorm ----------------
            with (
                tc.tile_pool(name="p3", bufs=2) as p3,
                tc.tile_pool(name="p3ps", bufs=1, space="PSUM") as p3ps,
            ):
                for cs in range(0, NLOC, 512):
                    w = min(512, NLOC - cs)
                    sl = slice(cs, cs + w)
                    ps_a = p3ps.tile([D, 512], F32, tag="ps_a")
                    nc.tensor.matmul(ps_a[:, :w], lhsT=wm2_sb[:], rhs=hall[:, sl],
                                     start=True, stop=True)
                    agg = p3.tile([D, 512], F16, tag="agg")
                    nc.vector.tensor_copy(agg[:, :w], ps_a[:, :w])

                    ps_u1 = p3ps.tile([D, 512], F32, tag="ps_u1")
                    nc.tensor.matmul(ps_u1[:, :w], lhsT=wu1_sb[:], rhs=agg[:, :w],
                                     start=True, stop=True)
                    s1 = p3.tile([D, 512], F16, tag="s1")
                    nc.scalar.activation(s1[:, :w], ps_u1[:, :w],
                                         mybir.ActivationFunctionType.Silu,
                                         bias=bu1_sb[:])
                    ps_u2 = p3ps.tile([D, 512], F32, tag="ps_u2")
                    nc.tensor.matmul(ps_u2[:, :w], lhsT=wu2_sb[:], rhs=s1[:, :w],
                                     start=True, stop=True)

                    nfl = p3.tile([D, 512], F32, tag="nfl")
                    nc.sync.dma_start(nfl[:, :w], nf_loc[:, sl])
                    xr = p3.tile([D, 512], F32, tag="xr")
                    nc.vector.scalar_tensor_tensor(
                        out=xr[:, :w], in0=ps_u2[:, :w], scalar=bu2_sb[:],
                        in1=nfl[:, :w],
                        op0=mybir.AluOpType.add, op1=mybir.AluOpType.add)

                    sq = p3.tile([D, 512], F32, tag="sq")
                    nc.scalar.activation(sq[:, :w], xr[:, :w],
                                         mybir.ActivationFunctionType.Square)
                    ps_s1 = p3ps.tile([1, 512], F32, tag="ps_s1")
                    nc.tensor.matmul(ps_s1[:, :w], lhsT=oinv[:], rhs=xr[:, :w],
                                     start=True, stop=True)
                    ps_s2 = p3ps.tile([1, 512], F32, tag="ps_s2")
                    nc.tensor.matmul(ps_s2[:, :w], lhsT=oinv[:], rhs=sq[:, :w],
                                     start=True, stop=True)
                    mean_sb = p3.tile([1, 512], F32, tag="mean_sb")
                    nc.vector.tensor_copy(mean_sb[:, :w], ps_s1[:, :w])
                    msq = p3.tile([1, 512], F32, tag="msq")
                    nc.vector.tensor_mul(msq[:, :w], mean_sb[:, :w], mean_sb[:, :w])
                    var = p3.tile([1, 512], F32, tag="var")
                    nc.vector.tensor_tensor(out=var[:, :w], in0=ps_s2[:, :w],
                                            in1=msq[:, :w],
                                            op=mybir.AluOpType.subtract)
                    std = p3.tile([1, 512], F32, tag="std")
                    nc.scalar.activation(std[:, :w], var[:, :w],
                                         mybir.ActivationFunctionType.Sqrt,
                                         bias=eps_sb[:])
                    rstd = p3.tile([1, 512], F32, tag="rstd")
                    nc.vector.reciprocal(rstd[:, :w], std[:, :w])

                    ps_mb = p3ps.tile([D, 512], F32, tag="ps_mb")
                    nc.tensor.matmul(ps_mb[:, :w], lhsT=ones1[:],
                                     rhs=mean_sb[:, :w], start=True, stop=True)
                    ps_rb = p3ps.tile([D, 512], F32, tag="ps_rb")
                    nc.tensor.matmul(ps_rb[:, :w], lhsT=ones1[:],
                                     rhs=rstd[:, :w], start=True, stop=True)

                    t1_ = p3.tile([D, 512], F32, tag="t1")
                    nc.vector.tensor_tensor(out=t1_[:, :w], in0=xr[:, :w],
                                            in1=ps_mb[:, :w],
                                            op=mybir.AluOpType.subtract)
                    t2_ = p3.tile([D, 512], F32, tag="t2")
                    nc.vector.tensor_mul(t2_[:, :w], t1_[:, :w], ps_rb[:, :w])
                    oc = p3.tile([D, 512], F32, tag="oc")
                    nc.scalar.activation(oc[:, :w], t2_[:, :w],
                                         mybir.ActivationFunctionType.Identity,
                                         bias=bet_sb[:], scale=gam_sb[:])
                    nc.sync.dma_start(out_fm[:, sl], oc[:, :w])

    nc.compile()
    return nc


def _wrap_idx(v):
    """flat slot-ordered indices -> [128, len/16] int16, replicated 8x."""
    w = v.reshape(-1, 16).T.astype(np.int16)      # [16, n/16]
    return np.ascontiguousarray(np.tile(w, (8, 1)))


def _prep(node_feat, edge_src, edge_dst, edge_feat,
          W_m1, b_m1, W_m2, b_m2, W_u1, b_u1, W_u2, b_u2,
          ln_gamma, ln_beta):
    """Host-side sharding: bucket edges by dst block, split each block's
    edges into lo/hi src halves, pad each half to whole 128-edge tiles."""
    order = np.argsort(edge_dst, kind="stable")
    sdst = edge_dst[order]

    # ---- per (core, block): edge lists split by src range ----
    per_cb = [[None] * BLOCKS for _ in range(NC)]
    for c in range(NC):
        lo_n, hi_n = c * NPC, (c + 1) * NPC
        e0, e1 = np.searchsorted(sdst, lo_n), np.searchsorted(sdst, hi_n)
        eidx = order[e0:e1]
        ldst = sdst[e0:e1] - lo_n
        bstart = np.searchsorted(ldst, np.arange(BLOCKS + 1) * 128)
        for b in range(BLOCKS):
            sel = eidx[bstart[b]:bstart[b + 1]]
            rel = ldst[bstart[b]:bstart[b + 1]] - b * 128
            src = edge_src[sel]
            m = src < SPLIT
            per_cb[c][b] = (sel[m], rel[m], sel[~m], rel[~m])

    lo_t = [1] * BLOCKS
    hi_t = [0] * BLOCKS
    for b in range(BLOCKS):
        for c in range(NC):
            slo, _, shi, _ = per_cb[c][b]
            lo_t[b] = max(lo_t[b], (len(slo) + 127) // 128)
            hi_t[b] = max(hi_t[b], (len(shi) + 127) // 128)
    nt = [l + h for l, h in zip(lo_t, hi_t)]
    T_tot = sum(nt)
    TB = np.concatenate([[0], np.cumsum(nt)]).astype(int)
    LO = np.concatenate([[0], np.cumsum(lo_t)]).astype(int)
    HI = np.concatenate([[0], np.cumsum(hi_t)]).astype(int)
    n_lo, n_hi = int(LO[-1]), int(HI[-1])

    nf_tb = np.zeros((65, NPAD), np.float16)
    nf_tb[0:D, 0:N_NODES] = node_feat.T
    nf_tb[64, :] = 1.0

    w_a = np.zeros((65, D), np.float16)
    w_a[0:D] = W_m1[0:D]
    w_a[64] = b_m1
    w_b = np.ascontiguousarray(W_m1[D:2 * D]).astype(np.float16)
    w_e = np.ascontiguousarray(W_m1[2 * D:3 * D]).astype(np.float16)
    w_m2e = np.zeros((65, D), np.float16)
    w_m2e[0:D] = W_m2
    w_m2e[64] = b_m2
    iota = np.tile(np.arange(128, dtype=np.float16), (128, 1))

    common = {
        "nf_tb": nf_tb, "iota": iota, "w_a": w_a, "w_b": w_b, "w_e": w_e,
        "w_m2e": w_m2e, "w_u1": W_u1.astype(np.float16),
        "b_u1": b_u1.reshape(D, 1).astype(np.float32),
        "w_u2": W_u2.astype(np.float16),
        "b_u2": b_u2.reshape(D, 1).astype(np.float32),
        "gam": ln_gamma.reshape(D, 1).astype(np.float32),
        "bet": ln_beta.reshape(D, 1).astype(np.float32),
    }

    in_maps = []
    for c in range(NC):
        lo_n = c * NPC
        src_lo = np.zeros(n_lo * 128, np.int64)
        src_hi = np.zeros(max(n_hi, 1) * 128, np.int64)
        dst_all = np.zeros(T_tot * 128, np.int64)
        dstrm = np.full((128, T_tot), -1.0, np.float16)
        ef_slots = np.zeros((T_tot * 128, D), np.float16)
        degc = np.zeros((1, NLOC), np.float16)

        for b in range(BLOCKS):
            slo, rlo, shi, rhi = per_cb[c][b]
            # lo region: tiles TB[b].., gather slots LO[b]*128..
            k = np.arange(len(slo))
            t_of = TB[b] + k // 128
            p_of = k % 128
            dstrm[p_of, t_of] = rlo.astype(np.float16)
            ef_slots[t_of * 128 + p_of] = edge_feat[slo]
            src_lo[LO[b] * 128 + k] = edge_src[slo]
            dst_all[TB[b] * 128 + k] = b * 128 + rlo
            # hi region: tiles TB[b]+lo_t[b].., gather slots HI[b]*128..
            k = np.arange(len(shi))
            t_of = TB[b] + lo_t[b] + k // 128
            p_of = k % 128
            dstrm[p_of, t_of] = rhi.astype(np.float16)
            ef_slots[t_of * 128 + p_of] = edge_feat[shi]
            src_hi[HI[b] * 128 + k] = edge_src[shi] - SPLIT
            dst_all[(TB[b] + lo_t[b]) * 128 + k] = b * 128 + rhi

            cnt = np.bincount(np.concatenate([rlo, rhi]).astype(np.int64),
                              minlength=128)
            degc[0, b * 128:(b + 1) * 128] = cnt[:128].astype(np.float16)

        nf_locc = np.zeros((D, NLOC), np.float32)
        nhi_node = min(N_NODES, lo_n + NLOC)
        nf_locc[:, 0:nhi_node - lo_n] = node_feat[lo_n:nhi_node].T

        in_maps.append({
            **common,
            "nf_loc": nf_locc,
            "nf_locb": nf_locc.astype(np.float16),
            "ef_t": np.ascontiguousarray(ef_slots.T),
            "idx_lo": _wrap_idx(src_lo),
            "idx_hi": _wrap_idx(src_hi),
            "idx_b": _wrap_idx(dst_all),
            "dstr": dstrm, "deg": degc,
        })
    return in_maps, (tuple(lo_t), tuple(hi_t))


def kernel(**inputs):
    inputs = {k: np.asarray(v) for k, v in inputs.items()}
    in_maps, struct = _prep(**inputs)
    if _CACHED.get("struct") != struct:
        _CACHED["nc"] = _build_bass(list(struct[0]), list(struct[1]))
        _CACHED["struct"] = struct
    res = run_bass_kernel_spmd(_CACHED["nc"], in_maps, list(range(NC)))
    out = np.empty((N_NODES, D), np.float32)
    for c in range(NC):
        out[c * NPC:(c + 1) * NPC] = res.results[c]["out_fm"].T[0:NPC]
    return out


if __name__ == "__main__":
    sys.path.insert(0, "/root/problem")
    import reference
    inputs = {k: np.asarray(v) for k, v in reference.setup_inputs().items()}
    exp = np.asarray(reference.reference(**inputs))
    got = kernel(**inputs)
    err = np.abs(got - exp).max() / (np.abs(exp).max() + 1e-30)
    print("Relative error:", err)
